# revision 4
# baseline (speedup 1.0000x reference)
"""Binary-weight 3x3 conv (BinaryConv2d) Trainium2 Bass kernel.

Reference computation (for x[32,256,56,56] f32, w[256,256,3,3] f32, b[256] f32):
    out = conv2d(x, sign(w), pad=1) + sign(b)[None,:,None,None]

Strategy:
  - Data-parallel over batch: 8 cores x 4 images each. No collectives.
  - Conv expressed as 9 shifted matmuls (taps) over a zero-padded SBUF image,
    contracting input channels (2 chunks of 128) into PSUM.
  - x is split into bf16 hi + bf16 lo (lo = x - hi, exact by Sterbenz), and both
    passes accumulate into the same PSUM tile -> fp32-grade accuracy at bf16
    PE throughput (weights are exactly +-1 in bf16).
  - Weights are binarized on-chip (ACT Sign) and transposed per-tap via the PE
    transpose path into [in_ch, out_ch] stationary tiles.
  - Output: PSUM -> SBUF via ACT Identity with per-partition binarized bias,
    then DMA to DRAM.
"""

from contextlib import ExitStack

import numpy as np

import concourse.bacc as bacc
import concourse.bass as bass
import concourse.tile as tile
import concourse.mybir as mybir
from concourse import masks
from concourse.bass_utils import run_bass_kernel_spmd

F32 = mybir.dt.float32
BF16 = mybir.dt.bfloat16

N_CORES = 8
B, C, H, W = 32, 256, 56, 56
O = 256
KH = KW = 3
BPC = B // N_CORES  # images per core

# padded SBUF image: rows 0..57 (vertical zero pad), cols 0..57 used (zero col 0
# and 57), width padded to 60 for tap slicing head-room
PH, PW = H + 2, 60
ROWS_PER_TILE = 8          # output rows per PSUM tile (8*56 = 448 <= 512 free)
N_ROW_CHUNKS = H // ROWS_PER_TILE
KI = C // 128              # input-channel chunks (contraction)
OC = O // 128              # output-channel chunks


def build_program(bpc=BPC, h=H, w=W, repeat=1):
    """Build the per-core Bass program. Returns compiled nc."""
    ph, pw = h + 2, w + 4
    n_row_chunks = max(1, h // ROWS_PER_TILE)
    rows = h // n_row_chunks

    nc = bacc.Bacc("TRN2", target_bir_lowering=False, debug=False,
                   num_devices=N_CORES)
    x_d = nc.dram_tensor("x", [bpc, C, h, w], F32, kind="ExternalInput").ap()
    w_d = nc.dram_tensor("weight", [O, C, KH, KW], F32,
                         kind="ExternalInput").ap()
    b_d = nc.dram_tensor("bias", [O], F32, kind="ExternalInput").ap()
    o_d = nc.dram_tensor("out", [bpc, O, h, w], F32, kind="ExternalOutput").ap()

    with tile.TileContext(nc) as tc, ExitStack() as ctx:
        const = ctx.enter_context(tc.tile_pool(name="const", bufs=1))
        wstg_p = ctx.enter_context(tc.tile_pool(name="wstg", bufs=2))
        xstg_p = ctx.enter_context(tc.tile_pool(name="xstg", bufs=3))
        xpad_p = ctx.enter_context(tc.tile_pool(name="xpad", bufs=2))
        out_p = ctx.enter_context(tc.tile_pool(name="outp", bufs=4))
        psum_p = ctx.enter_context(
            tc.tile_pool(name="psum", bufs=4, space=bass.MemorySpace.PSUM))
        tpsum_p = ctx.enter_context(
            tc.tile_pool(name="tpsum", bufs=2, space=bass.MemorySpace.PSUM))

        # ---- constants ----
        identity = const.tile([128, 128], BF16)
        masks.make_identity(nc, identity[:])

        bias_raw = const.tile([128, OC], F32)
        bias_bin = const.tile([128, OC], F32)
        # bias_raw[p, oc] = bias[oc*128 + p]
        nc.sync.dma_start(out=bias_raw[:],
                          in_=b_d.rearrange("(b a) -> a b", b=OC))
        nc.scalar.sign(bias_bin[:], bias_raw[:])

        # ---- weights: load, binarize, transpose per tap ----
        # lhsT_all[:, idx, :] = sign(W[oc_chunk, ki_chunk, tap]).T  (shape [i,o])
        lhsT_all = const.tile([128, KI * KH * KW * OC, 128], BF16)

        def lidx(ki, ky, kx, oc):
            return ((ki * KH + ky) * KW + kx) * OC + oc

        for ki in range(KI):
            for oc in range(OC):
                wstg = wstg_p.tile([128, 128, KH, KW], F32, tag="wstg")
                nc.sync.dma_start(
                    out=wstg[:],
                    in_=w_d[oc * 128:(oc + 1) * 128, ki * 128:(ki + 1) * 128, :, :])
                wbin = wstg_p.tile([128, 128, KH, KW], BF16, tag="wbin")
                nc.scalar.sign(wbin[:], wstg[:])
                for ky in range(KH):
                    for kx in range(KW):
                        tp = tpsum_p.tile([128, 128], BF16)
                        nc.tensor.transpose(tp[:], wbin[:, :, ky, kx], identity[:])
                        nc.vector.tensor_copy(
                            lhsT_all[:, lidx(ki, ky, kx, oc), :], tp[:])

        # ---- main loop over images ----
        for _rep in range(repeat):
            for n in range(bpc):
                xpad = {}
                for ki in range(KI):
                    xf = xstg_p.tile([128, h, w], F32, tag="xf")
                    nc.sync.dma_start(out=xf[:],
                                      in_=x_d[n, ki * 128:(ki + 1) * 128, :, :])
                    hi = xpad_p.tile([128, ph, pw], BF16, tag=f"hi{ki}")
                    lo = xpad_p.tile([128, ph, pw], BF16, tag=f"lo{ki}")
                    for t in (hi, lo):
                        nc.gpsimd.memset(t[:, 0, :], 0.0)
                        nc.gpsimd.memset(t[:, ph - 1, :], 0.0)
                        nc.gpsimd.memset(t[:, 1:ph - 1, 0], 0.0)
                        nc.gpsimd.memset(t[:, 1:ph - 1, w + 1:pw], 0.0)
                    # hi = bf16(x)
                    nc.scalar.copy(hi[:, 1:h + 1, 1:w + 1], xf[:])
                    # lo = bf16(x - f32(hi))
                    hif = xstg_p.tile([128, h, w], F32, tag="hif")
                    nc.scalar.copy(hif[:], hi[:, 1:h + 1, 1:w + 1])
                    nc.vector.tensor_sub(lo[:, 1:h + 1, 1:w + 1], xf[:], hif[:])
                    xpad[("hi", ki)] = hi
                    xpad[("lo", ki)] = lo

                for oc in range(OC):
                    for rc in range(n_row_chunks):
                        r0 = rc * rows
                        ps = psum_p.tile([128, rows, w], F32)
                        k = 0
                        nmm = 2 * KI * KH * KW
                        for p in ("hi", "lo"):
                            for ki in range(KI):
                                xp = xpad[(p, ki)]
                                for ky in range(KH):
                                    for kx in range(KW):
                                        nc.tensor.matmul(
                                            ps[:],
                                            lhsT_all[:, lidx(ki, ky, kx, oc), :],
                                            xp[:, r0 + ky:r0 + ky + rows,
                                               kx:kx + w],
                                            start=(k == 0),
                                            stop=(k == nmm - 1))
                                        k += 1
                        ob = out_p.tile([128, rows, w], F32)
                        nc.scalar.activation(
                            ob[:], ps[:], mybir.ActivationFunctionType.Identity,
                            bias=bias_bin[:, oc:oc + 1], scale=1.0)
                        nc.sync.dma_start(
                            out=o_d[n, oc * 128:(oc + 1) * 128, r0:r0 + rows, :],
                            in_=ob[:])

    nc.compile()
    return nc


_CACHE = {}


def _get_program():
    if "nc" not in _CACHE:
        _CACHE["nc"] = build_program()
    return _CACHE["nc"]


def kernel(x, weight, bias):
    x = np.ascontiguousarray(x, dtype=np.float32)
    weight = np.ascontiguousarray(weight, dtype=np.float32)
    bias = np.ascontiguousarray(bias, dtype=np.float32)
    nc = _get_program()
    in_maps = [
        {"x": x[c * BPC:(c + 1) * BPC], "weight": weight, "bias": bias}
        for c in range(N_CORES)
    ]
    r = run_bass_kernel_spmd(nc, in_maps, list(range(N_CORES)))
    return np.concatenate([r.results[c]["out"] for c in range(N_CORES)], axis=0)
